# revision 20
# baseline (speedup 1.0000x reference)
"""Trainium2 Bass kernel for DistanceTransformLayer2.

Reference semantics (B=8, C=1, H=W=256):
    D_i[h,w] = sqrt(h^2 + (i-w)^2)
    out[b,c,i,j] = max_{h,w} -(D_i[h,w] + f[b,c,h,w])   for even j
    out[b,c,i,j] = max_{h,w} D_i[h,w]                   for odd  j
                 = sqrt(255^2 + max(i,255-i)^2)         (input-independent)

Key algebraic facts used:
  * D_i[h,w] depends only on (h, i-w): D_i[h,w] = g[h,d] with
    d = (i-w)+(R-1) and g[h,d] = sqrt(h^2+(d-(R-1))^2) >= max(h,|d-(R-1)|).
  * Window pruning bound (exact, data-dependent radius R chosen on host):
    since (h=0,w=i) is in the window, the window min of (D+f) is <= fmax.
    Every point outside {h<R, |i-w|<=R-1} has D >= R, so its value is
    >= R + fmin.  Hence for any R >= fmax - fmin the window min equals
    the global min EXACTLY.  R = ceil(fmax-fmin)+1 (~11 for N(0,1)).

Sharding: data-parallel over batch B — core b computes batch b.

Device layout per core (fast path, R <= 88):
  i sits on PARTITIONS (i_lo = i & 127, two ih halves), and the whole
  (h,d) window sits on the FREE axis, im2col-style, packed by the host:
  pk[i_lo, RW+4+ih*RW + h*WIN+d] = f[h, i-(R-1)+d] (PAD outside).
  The g table (cols [0,RW)) and the odd-column constants (cols RW..RW+4,
  interleaved with the accumulator slots) ride in the same tensor, so
  TWO input DMAs (sync + scalar HWDGE rings, in parallel) load
  everything.  One fused DVE tensor_tensor_reduce per ih half computes
      acc[i] = max_{h,d} -(f + g)
  directly into the cm slot next to the host-shipped odd constant; one
  broadcast copy per ih (gpsimd / vector) interleaves (even,odd) into
  the output tile, and two HWDGE DMAs (sync + scalar) store the halves.
"""

import numpy as np

_H = 256
_W = 256
_B = 8
_N_CORES = 8
_PAD = np.float32(1.0e30)
_R_FAST_MAX = 88

_KERNEL_CACHE = {}


def _build_bass_fast(R):
    import concourse.bacc as bacc
    import concourse.bass as bass
    import concourse.mybir as mybir
    from concourse.tile import TileContext

    WIN = 2 * R - 1
    RW = R * WIN

    nc = bacc.Bacc("TRN2", target_bir_lowering=False, debug=False,
                   num_devices=_N_CORES)
    dt = mybir.dt.float32
    bf = mybir.dt.bfloat16
    # pk columns (bf16): [0,RW) g table | [RW,2RW) ih0 win | [2RW,3RW) ih1
    pk_in = nc.dram_tensor("pk", [128, 3 * RW], bf,
                           kind="ExternalInput").ap()
    moddt_in = nc.dram_tensor("moddt", [128, 2], dt,
                              kind="ExternalInput").ap()
    out_ext = nc.dram_tensor("out", [_H, _W], dt, kind="ExternalOutput").ap()

    AluOp = mybir.AluOpType

    with TileContext(nc) as tc:
        with (
            tc.tile_pool(name="work", bufs=1) as work,
        ):
            gfab = work.tile([128, 3 * RW], bf, tag="gfab")
            scratch = work.tile([128, 2 * RW], bf, tag="scratch")
            cm = work.tile([128, 4], dt, tag="cm")
            outt = work.tile([128, 2 * _W], dt, tag="outt")

            # [g2|fa] on the sync HWDGE ring, [fb] on the scalar ring;
            # the tiny odd-column consts follow on the scalar ring
            nc.sync.dma_start(out=gfab[:, 0:2 * RW], in_=pk_in[:, 0:2 * RW])
            nc.scalar.dma_start(out=gfab[:, 2 * RW:3 * RW],
                                in_=pk_in[:, 2 * RW:3 * RW])
            cm_ap = cm[:]
            nc.scalar.dma_start(out=cm[:, 2:4], in_=moddt_in[:])

            # acc[i] = -min_{h,d} (f + g), both ih halves in one ADD+MIN
            # (tensor_tensor_reduce would fuse these but its ISA encoding
            # wedges the HW on this runtime)
            g_ap = gfab[:]
            in1 = bass.AP(tensor=g_ap.tensor, offset=g_ap.offset,
                          ap=[list(g_ap.ap[0]), [0, 2], [1, RW]])
            sc3 = scratch[:].rearrange("p (i x) -> p i x", x=RW)
            nc.vector.tensor_tensor(
                out=sc3, in0=gfab[:, RW:3 * RW].rearrange(
                    "p (i x) -> p i x", x=RW),
                in1=in1, op=AluOp.add)
            nc.vector.tensor_reduce(
                out=cm[:, 0:2], in_=sc3, axis=mybir.AxisListType.X,
                op=AluOp.min, negate=True)

            # interleave (even=acc col ih, odd=const col 2+ih) into outt;
            # ih0 on DVE (follows the reduce in-engine), ih1 on Activation
            # so the two copies and the two out-DMA gens overlap
            o_ap = outt[:]
            for ih in range(2):
                src = bass.AP(tensor=cm_ap.tensor,
                              offset=cm_ap.offset + ih,
                              ap=[list(cm_ap.ap[0]), [0, _W // 2], [2, 2]])
                dst = bass.AP(tensor=o_ap.tensor,
                              offset=o_ap.offset + ih * _W,
                              ap=[list(o_ap.ap[0]), [2, _W // 2], [1, 2]])
                if ih == 0:
                    nc.vector.tensor_copy(dst, src)
                else:
                    nc.scalar.copy(dst, src)

            nc.sync.dma_start(out=out_ext[0:128, :], in_=outt[:, 0:_W])
            nc.scalar.dma_start(out=out_ext[128:256, :],
                                in_=outt[:, _W:2 * _W])

    nc.compile()
    return nc


def _pack_fast(f_b, R, gtab_row):
    """Host-side im2col pack for one batch. f_b: [H, W] fp32 -> bf16."""
    import ml_dtypes
    bf16 = ml_dtypes.bfloat16
    WIN = 2 * R - 1
    RW = R * WIN
    W2 = _W + 2 * (R - 1)
    fw = np.full((R, W2), _PAD, np.float32).astype(bf16)
    fw[:, R - 1:R - 1 + _W] = f_b[:R, :].astype(bf16)
    s0, s1 = fw.strides
    win = np.lib.stride_tricks.as_strided(
        fw, shape=(_H, R, WIN), strides=(s1, s0, s1))
    win2 = win.reshape(_H, RW)
    pk = np.empty((128, 3 * RW), bf16)
    pk[:, 0:RW] = gtab_row.astype(bf16)
    # partition p computes output rows i = p (ih=0) and p+128 (ih=1)
    pk[:, RW:2 * RW] = win2[0:128]
    pk[:, 2 * RW:3 * RW] = win2[128:256]
    return pk


# ---------------------------------------------------------------------------
# Fallback path (original kernel) for large R — partitions pack (j, ih, h),
# vector add+min against a replicated g table, PE-transpose, reduce, copy.
# ---------------------------------------------------------------------------

def _params(R):
    if R <= 32:
        G, HP = 4, 32
    else:
        G, HP = 2, 64
    NHT = -(-R // HP)          # h tiles (1 unless R > 64)
    NG = G // 2                # transpose chunks
    IW = 256 // G              # i width per block
    WIN = 2 * R - 1
    PW = IW + 2 * (R - 1)      # fpk free width per block
    W2 = 256 + 2 * (R - 1)     # host fwin width
    IC = IW
    while IC > 1 and IC * WIN > 16384:
        IC //= 2
    return G, HP, NHT, NG, IW, WIN, PW, W2, IC


def _build_bass(R):
    import concourse.bacc as bacc
    import concourse.bass as bass
    import concourse.mybir as mybir
    from concourse.tile import TileContext

    G, HP, NHT, NG, IW, WIN, PW, W2, IC = _params(R)
    NP = G * HP                # partitions in use (<= 128)
    NIC = IW // IC

    nc = bacc.Bacc("TRN2", target_bir_lowering=False, debug=False,
                   num_devices=_N_CORES)
    dt = mybir.dt.float32
    fwin_in = nc.dram_tensor("fwin", [NHT * 128, PW + WIN], dt,
                             kind="ExternalInput").ap()
    moddt_in = nc.dram_tensor("moddt", [128, 2], dt,
                              kind="ExternalInput").ap()
    ident_in = nc.dram_tensor("ident", [NG * 2 * HP, 2 * HP], dt,
                              kind="ExternalInput").ap()
    out_ext = nc.dram_tensor("out", [_H, _W], dt, kind="ExternalOutput").ap()

    AluOp = mybir.AluOpType

    with TileContext(nc) as tc:
        with (
            tc.tile_pool(name="consts", bufs=1) as consts,
            tc.tile_pool(name="work", bufs=2) as work,
            tc.tile_pool(name="acc", bufs=1) as accp,
            tc.tile_pool(name="psum", bufs=1, space="PSUM") as psump,
        ):
            ident = consts.tile([NG * 2 * HP, 2 * HP], dt)
            nc.gpsimd.dma_start(out=ident[:], in_=ident_in[:])

            cm = consts.tile([128, 4], dt)
            cm_ap = cm[:]
            modd_dst = bass.AP(tensor=cm_ap.tensor, offset=cm_ap.offset + 1,
                               ap=[list(cm_ap.ap[0]), [2, 2]])
            nc.gpsimd.dma_start(out=modd_dst, in_=moddt_in[:])

            macc = accp.tile([NP, IW], dt)
            macc2 = accp.tile([NP, IW], dt)

            for ht in range(NHT):
                fpk = work.tile([NP, PW + WIN], dt, tag="fpk")
                nc.sync.dma_start(
                    out=fpk[:], in_=fwin_in[ht * 128:(ht + 1) * 128, :])
                gpk = fpk[:, PW:PW + WIN]

                for icc in range(NIC):
                    i0 = icc * IC
                    tmp = work.tile([NP, IC * WIN], dt, tag="tmp")
                    fpk_ap = fpk[:]
                    in0 = bass.AP(
                        tensor=fpk_ap.tensor,
                        offset=fpk_ap.offset + i0,
                        ap=[list(fpk_ap.ap[0]), [1, IC], [1, WIN]],
                    )
                    in1 = gpk[:, None, :].broadcast_to([NP, IC, WIN])
                    tmp3 = tmp[:].rearrange("p (i d) -> p i d", d=WIN)
                    nc.vector.tensor_tensor(out=tmp3, in0=in0, in1=in1,
                                            op=AluOp.add)
                    dst = macc if ht == 0 else macc2
                    nc.vector.tensor_reduce(
                        out=dst[:, i0:i0 + IC], in_=tmp3,
                        axis=mybir.AxisListType.X, op=AluOp.min,
                    )
                if ht > 0:
                    nc.vector.tensor_tensor(out=macc[:], in0=macc[:],
                                            in1=macc2[:], op=AluOp.min)

            pt = psump.tile([128, 2 * HP], dt)
            for j in range(NG):
                nc.tensor.matmul(
                    pt[j * IW:(j + 1) * IW, :],
                    macc[j * 2 * HP:(j + 1) * 2 * HP, :],
                    ident[j * 2 * HP:(j + 1) * 2 * HP, :],
                    start=True, stop=True,
                )

            cm_ev = bass.AP(tensor=cm_ap.tensor, offset=cm_ap.offset,
                            ap=[list(cm_ap.ap[0]), [2, 2]])
            pt_ap = pt[:]
            pt3 = bass.AP(tensor=pt_ap.tensor, offset=pt_ap.offset,
                          ap=[list(pt_ap.ap[0]), [HP, 2], [1, HP]])
            nc.vector.tensor_reduce(out=cm_ev, in_=pt3,
                                    axis=mybir.AxisListType.X,
                                    op=AluOp.min, negate=True)

            for ih in range(2):
                outt = work.tile([128, _W], dt, tag="outt")
                src = bass.AP(tensor=cm_ap.tensor,
                              offset=cm_ap.offset + 2 * ih,
                              ap=[list(cm_ap.ap[0]), [0, _W // 2], [1, 2]])
                outt_ap = outt[:]
                dst = bass.AP(tensor=outt_ap.tensor, offset=outt_ap.offset,
                              ap=[list(outt_ap.ap[0]), [2, _W // 2], [1, 2]])
                nc.vector.tensor_copy(dst, src)
                eng = nc.sync if ih == 0 else nc.scalar
                eng.dma_start(out=out_ext[ih * 128:(ih + 1) * 128, :],
                              in_=outt[:])

    nc.compile()
    return nc


def _get_bass(R):
    key = ("fast", R) if R <= _R_FAST_MAX else ("slow", R)
    if key not in _KERNEL_CACHE:
        _KERNEL_CACHE[key] = (_build_bass_fast(R) if key[0] == "fast"
                              else _build_bass(R))
    return _KERNEL_CACHE[key]


def _modd_vec():
    ii = np.arange(_H)
    return np.sqrt(
        np.float32(255.0) ** 2
        + np.maximum(ii, 255 - ii).astype(np.float32) ** 2
    ).astype(np.float32)


def kernel(feature_map, feature_size=None, **_unused):
    from concourse.bass_utils import run_bass_kernel_spmd

    f = np.ascontiguousarray(np.asarray(feature_map, dtype=np.float32))
    assert f.shape == (_B, 1, _H, _W), f.shape

    fmax = float(f.max())
    fmin = float(f.min())
    R = int(np.ceil(fmax - fmin)) + 1
    R = max(2, min(R, _H))

    modd = _modd_vec()
    nc = _get_bass(R)

    if R <= _R_FAST_MAX:
        WIN = 2 * R - 1
        hh = np.arange(R, dtype=np.float32)
        dd = np.arange(-(R - 1), R, dtype=np.float32)
        gtab = np.sqrt(hh[:, None] ** 2 + dd[None, :] ** 2).astype(np.float32)
        gtab_row = gtab.reshape(1, R * WIN)
        moddt = np.ascontiguousarray(modd.reshape(2, 128).T)
        in_maps = [{"pk": _pack_fast(f[b, 0], R, gtab_row), "moddt": moddt}
                   for b in range(_B)]
    else:
        G, HP, NHT, NG, IW, WIN, PW, W2, IC = _params(R)
        hh = np.arange(NHT * HP, dtype=np.float32)
        dd = np.arange(-(R - 1), R, dtype=np.float32)
        gtab = np.sqrt(hh[:, None] ** 2 + dd[None, :] ** 2).astype(np.float32)
        gtab[R:, :] = 0.0
        gdup = np.concatenate([np.tile(gtab[t * HP:(t + 1) * HP], (G, 1))
                               for t in range(NHT)], axis=0)
        moddt = np.ascontiguousarray(modd.reshape(2, 128).T)
        ident = np.ascontiguousarray(
            np.tile(np.eye(2 * HP, dtype=np.float32), (NG, 1)))
        in_maps = []
        for b in range(_B):
            fw = np.full((NHT * HP, W2), _PAD, np.float32)
            fw[:R, R - 1:R - 1 + _W] = f[b, 0, :R, :]
            fpk = np.empty((NHT, 128, PW + WIN), np.float32)
            for j in range(NG):
                for ih in range(2):
                    ib = ih * NG + j
                    p0 = j * 2 * HP + ih * HP
                    for t in range(NHT):
                        fpk[t, p0:p0 + HP, :PW] = \
                            fw[t * HP:(t + 1) * HP, ib * IW:ib * IW + PW]
            fpk[:, :, PW:] = gdup.reshape(NHT, 128, WIN)
            fpk = np.ascontiguousarray(fpk.reshape(NHT * 128, PW + WIN))
            in_maps.append({"fwin": fpk, "moddt": moddt, "ident": ident})

    res = run_bass_kernel_spmd(nc, in_maps, list(range(_N_CORES)))
    out = np.stack([res.results[b]["out"] for b in range(_B)])[:, None]
    return np.ascontiguousarray(out.astype(np.float32))


# revision 25
# speedup vs baseline: 1.1145x; 1.1145x over previous
"""Trainium2 Bass kernel for DistanceTransformLayer2.

Reference semantics (B=8, C=1, H=W=256):
    D_i[h,w] = sqrt(h^2 + (i-w)^2)
    out[b,c,i,j] = max_{h,w} -(D_i[h,w] + f[b,c,h,w])   for even j
    out[b,c,i,j] = max_{h,w} D_i[h,w]                   for odd  j
                 = sqrt(255^2 + max(i,255-i)^2)         (input-independent)

Key algebraic facts used:
  * D_i[h,w] depends only on (h, i-w): D_i[h,w] = g[h,d] with
    d = (i-w)+(R-1) and g[h,d] = sqrt(h^2+(d-(R-1))^2) >= max(h,|d-(R-1)|).
  * Window pruning bound (exact, data-dependent radius R chosen on host):
    since (h=0,w=i) is in the window, the window min of (D+f) is <= fmax.
    Every point outside {h<R, |i-w|<=R-1} has D >= R, so its value is
    >= R + fmin.  Hence for any R >= fmax - fmin the window min equals
    the global min EXACTLY.  R = ceil(fmax-fmin)+1 (~11 for N(0,1)).

Sharding: data-parallel over batch B — core b computes batch b.

Device layout per core (fast path, R <= 88):
  i sits on PARTITIONS (i_lo = i & 127, two ih halves), and the whole
  (h,d) window sits on the FREE axis, im2col-style, packed by the host:
  pk[i_lo, RW+4+ih*RW + h*WIN+d] = f[h, i-(R-1)+d] (PAD outside).
  The g table (cols [0,RW)) and the odd-column constants (cols RW..RW+4,
  interleaved with the accumulator slots) ride in the same tensor, so
  TWO input DMAs (sync + scalar HWDGE rings, in parallel) load
  everything.  One fused DVE tensor_tensor_reduce per ih half computes
      acc[i] = max_{h,d} -(f + g)
  directly into the cm slot next to the host-shipped odd constant; one
  broadcast copy per ih (gpsimd / vector) interleaves (even,odd) into
  the output tile, and two HWDGE DMAs (sync + scalar) store the halves.
"""

import numpy as np

_H = 256
_W = 256
_B = 8
_N_CORES = 8
_PAD = np.float32(1.0e30)
_R_FAST_MAX = 88

_KERNEL_CACHE = {}


def _build_bass_fast(R):
    import concourse.bacc as bacc
    import concourse.bass as bass
    import concourse.mybir as mybir
    from concourse.tile import TileContext

    WIN = 2 * R - 1
    RW = R * WIN

    nc = bacc.Bacc("TRN2", target_bir_lowering=False, debug=False,
                   num_devices=_N_CORES)
    dt = mybir.dt.float32
    bf = mybir.dt.bfloat16
    # pk columns (bf16): [0,RW) g table | [RW,2RW) ih0 win | [2RW,3RW) ih1
    pk_in = nc.dram_tensor("pk", [128, 3 * RW], bf,
                           kind="ExternalInput").ap()
    moddt_in = nc.dram_tensor("moddt", [128, 2], dt,
                              kind="ExternalInput").ap()
    out_ext = nc.dram_tensor("out", [_H, _W], dt, kind="ExternalOutput").ap()

    AluOp = mybir.AluOpType

    with TileContext(nc) as tc:
        with (
            tc.tile_pool(name="work", bufs=1) as work,
        ):
            gfab = work.tile([128, 3 * RW], bf, tag="gfab")
            scratch = work.tile([128, 2 * RW], bf, tag="scratch")
            cm = work.tile([128, 4], dt, tag="cm")
            outt = work.tile([128, 2 * _W], dt, tag="outt")

            # [g2|fa] on the sync HWDGE ring, [fb] on the scalar ring;
            # the tiny odd-column consts follow on the scalar ring
            nc.sync.dma_start(out=gfab[:, 0:2 * RW], in_=pk_in[:, 0:2 * RW])
            nc.scalar.dma_start(out=gfab[:, 2 * RW:3 * RW],
                                in_=pk_in[:, 2 * RW:3 * RW])
            cm_ap = cm[:]
            nc.scalar.dma_start(out=cm[:, 2:4], in_=moddt_in[:])

            # acc[i] = -min_{h,d} (f + g), both ih halves in one ADD+MIN
            # (tensor_tensor_reduce would fuse these but its ISA encoding
            # wedges the HW on this runtime)
            g_ap = gfab[:]
            in1 = bass.AP(tensor=g_ap.tensor, offset=g_ap.offset,
                          ap=[list(g_ap.ap[0]), [0, 2], [1, RW]])
            sc3 = scratch[:].rearrange("p (i x) -> p i x", x=RW)
            nc.vector.tensor_tensor(
                out=sc3, in0=gfab[:, RW:3 * RW].rearrange(
                    "p (i x) -> p i x", x=RW),
                in1=in1, op=AluOp.add)
            nc.vector.tensor_reduce(
                out=cm[:, 0:2], in_=sc3, axis=mybir.AxisListType.X,
                op=AluOp.min, negate=True)

            # interleave (even=acc col ih, odd=const col 2+ih) into outt;
            # ih0 on DVE (follows the reduce in-engine), ih1 on Activation
            # so the two copies and the two out-DMA gens overlap
            o_ap = outt[:]
            for ih in range(2):
                src = bass.AP(tensor=cm_ap.tensor,
                              offset=cm_ap.offset + ih,
                              ap=[list(cm_ap.ap[0]), [0, _W // 2], [2, 2]])
                dst = bass.AP(tensor=o_ap.tensor,
                              offset=o_ap.offset + ih * _W,
                              ap=[list(o_ap.ap[0]), [2, _W // 2], [1, 2]])
                if ih == 0:
                    nc.vector.tensor_copy(dst, src)
                else:
                    nc.scalar.copy(dst, src)

            nc.sync.dma_start(out=out_ext[0:128, :], in_=outt[:, 0:_W])
            nc.scalar.dma_start(out=out_ext[128:256, :],
                                in_=outt[:, _W:2 * _W])

    nc.compile()
    return nc


def _pack_fast(f_b, R, gtab_row):
    """Host-side im2col pack for one batch. f_b: [H, W] fp32 -> bf16."""
    import ml_dtypes
    bf16 = ml_dtypes.bfloat16
    WIN = 2 * R - 1
    RW = R * WIN
    W2 = _W + 2 * (R - 1)
    fw = np.full((R, W2), _PAD, np.float32).astype(bf16)
    fw[:, R - 1:R - 1 + _W] = f_b[:R, :].astype(bf16)
    s0, s1 = fw.strides
    win = np.lib.stride_tricks.as_strided(
        fw, shape=(_H, R, WIN), strides=(s1, s0, s1))
    win2 = win.reshape(_H, RW)
    pk = np.empty((128, 3 * RW), bf16)
    pk[:, 0:RW] = gtab_row.astype(bf16)
    # partition p computes output rows i = p (ih=0) and p+128 (ih=1)
    pk[:, RW:2 * RW] = win2[0:128]
    pk[:, 2 * RW:3 * RW] = win2[128:256]
    return pk


def _build_bass_raw(R):
    """Raw-bass variant of the fast path: same dataflow, manual semaphores,
    no TileContext (skips the tile scheduling prologue/epilogue)."""
    import concourse.bacc as bacc
    import concourse.bass as bass
    import concourse.mybir as mybir

    WIN = 2 * R - 1
    RW = R * WIN

    nc = bacc.Bacc("TRN2", target_bir_lowering=False, debug=False,
                   num_devices=_N_CORES)
    dt = mybir.dt.float32
    bf = mybir.dt.bfloat16
    AluOp = mybir.AluOpType
    pk_in = nc.dram_tensor("pk", [128, 3 * RW], bf,
                           kind="ExternalInput").ap()
    moddt_in = nc.dram_tensor("moddt", [128, 2], dt,
                              kind="ExternalInput").ap()
    out_ext = nc.dram_tensor("out", [_H, _W], dt, kind="ExternalOutput").ap()

    gfab = nc.alloc_sbuf_tensor("gfab", [128, 3 * RW], bf).ap()
    scratch = nc.alloc_sbuf_tensor("scratch", [128, 2 * RW], bf).ap()
    cm = nc.alloc_sbuf_tensor("cm", [128, 4], dt).ap()
    outt = nc.alloc_sbuf_tensor("outt", [128, 2 * _W], dt).ap()

    s_in = nc.alloc_semaphore("s_in")
    s_modd = nc.alloc_semaphore("s_modd")
    s_add = nc.alloc_semaphore("s_add")
    s_min = nc.alloc_semaphore("s_min")
    s_c0 = nc.alloc_semaphore("s_c0")
    s_c1 = nc.alloc_semaphore("s_c1")
    s_out = nc.alloc_semaphore("s_out")

    def cp_aps(ih):
        src = bass.AP(tensor=cm.tensor, offset=cm.offset + ih,
                      ap=[list(cm.ap[0]), [0, _W // 2], [2, 2]])
        dst = bass.AP(tensor=outt.tensor, offset=outt.offset + ih * _W,
                      ap=[list(outt.ap[0]), [2, _W // 2], [1, 2]])
        return src, dst

    with nc.Block("dt2") as blk:
        @blk.sync
        def _(sync):
            sync.dma_start(out=gfab[:, 0:2 * RW],
                           in_=pk_in[:, 0:2 * RW]).then_inc(s_in, 16)
            sync.wait_ge(s_c0, 1)
            sync.dma_start(out=out_ext[0:128, :],
                           in_=outt[:, 0:_W]).then_inc(s_out, 16)
            sync.wait_ge(s_out, 32)

        @blk.scalar
        def _(scalar):
            scalar.dma_start(out=gfab[:, 2 * RW:3 * RW],
                             in_=pk_in[:, 2 * RW:3 * RW]).then_inc(s_in, 16)
            scalar.dma_start(out=cm[:, 2:4],
                             in_=moddt_in[:]).then_inc(s_modd, 16)
            scalar.wait_ge(s_modd, 16)
            scalar.wait_ge(s_min, 1)
            src, dst = cp_aps(1)
            scalar.copy(dst, src).then_inc(s_c1, 1)
            scalar.wait_ge(s_c1, 1)
            scalar.dma_start(out=out_ext[128:256, :],
                             in_=outt[:, _W:2 * _W]).then_inc(s_out, 16)

        @blk.vector
        def _(vector):
            vector.wait_ge(s_in, 32)
            in1 = bass.AP(tensor=gfab.tensor, offset=gfab.offset,
                          ap=[list(gfab.ap[0]), [0, 2], [1, RW]])
            sc3 = scratch.rearrange("p (i x) -> p i x", x=RW)
            vector.tensor_tensor(
                out=sc3,
                in0=gfab[:, RW:3 * RW].rearrange("p (i x) -> p i x", x=RW),
                in1=in1, op=AluOp.add).then_inc(s_add, 1)
            vector.wait_ge(s_modd, 16)
            vector.wait_ge(s_add, 1)
            vector.tensor_reduce(
                out=cm[:, 0:2], in_=sc3, axis=mybir.AxisListType.X,
                op=AluOp.min, negate=True).then_inc(s_min, 1)
            vector.wait_ge(s_min, 1)
            src, dst = cp_aps(0)
            vector.tensor_copy(dst, src).then_inc(s_c0, 1)

    nc.compile()
    return nc


# ---------------------------------------------------------------------------
# Fallback path (original kernel) for large R — partitions pack (j, ih, h),
# vector add+min against a replicated g table, PE-transpose, reduce, copy.
# ---------------------------------------------------------------------------

def _params(R):
    if R <= 32:
        G, HP = 4, 32
    else:
        G, HP = 2, 64
    NHT = -(-R // HP)          # h tiles (1 unless R > 64)
    NG = G // 2                # transpose chunks
    IW = 256 // G              # i width per block
    WIN = 2 * R - 1
    PW = IW + 2 * (R - 1)      # fpk free width per block
    W2 = 256 + 2 * (R - 1)     # host fwin width
    IC = IW
    while IC > 1 and IC * WIN > 16384:
        IC //= 2
    return G, HP, NHT, NG, IW, WIN, PW, W2, IC


def _build_bass(R):
    import concourse.bacc as bacc
    import concourse.bass as bass
    import concourse.mybir as mybir
    from concourse.tile import TileContext

    G, HP, NHT, NG, IW, WIN, PW, W2, IC = _params(R)
    NP = G * HP                # partitions in use (<= 128)
    NIC = IW // IC

    nc = bacc.Bacc("TRN2", target_bir_lowering=False, debug=False,
                   num_devices=_N_CORES)
    dt = mybir.dt.float32
    fwin_in = nc.dram_tensor("fwin", [NHT * 128, PW + WIN], dt,
                             kind="ExternalInput").ap()
    moddt_in = nc.dram_tensor("moddt", [128, 2], dt,
                              kind="ExternalInput").ap()
    ident_in = nc.dram_tensor("ident", [NG * 2 * HP, 2 * HP], dt,
                              kind="ExternalInput").ap()
    out_ext = nc.dram_tensor("out", [_H, _W], dt, kind="ExternalOutput").ap()

    AluOp = mybir.AluOpType

    with TileContext(nc) as tc:
        with (
            tc.tile_pool(name="consts", bufs=1) as consts,
            tc.tile_pool(name="work", bufs=2) as work,
            tc.tile_pool(name="acc", bufs=1) as accp,
            tc.tile_pool(name="psum", bufs=1, space="PSUM") as psump,
        ):
            ident = consts.tile([NG * 2 * HP, 2 * HP], dt)
            nc.gpsimd.dma_start(out=ident[:], in_=ident_in[:])

            cm = consts.tile([128, 4], dt)
            cm_ap = cm[:]
            modd_dst = bass.AP(tensor=cm_ap.tensor, offset=cm_ap.offset + 1,
                               ap=[list(cm_ap.ap[0]), [2, 2]])
            nc.gpsimd.dma_start(out=modd_dst, in_=moddt_in[:])

            macc = accp.tile([NP, IW], dt)
            macc2 = accp.tile([NP, IW], dt)

            for ht in range(NHT):
                fpk = work.tile([NP, PW + WIN], dt, tag="fpk")
                nc.sync.dma_start(
                    out=fpk[:], in_=fwin_in[ht * 128:(ht + 1) * 128, :])
                gpk = fpk[:, PW:PW + WIN]

                for icc in range(NIC):
                    i0 = icc * IC
                    tmp = work.tile([NP, IC * WIN], dt, tag="tmp")
                    fpk_ap = fpk[:]
                    in0 = bass.AP(
                        tensor=fpk_ap.tensor,
                        offset=fpk_ap.offset + i0,
                        ap=[list(fpk_ap.ap[0]), [1, IC], [1, WIN]],
                    )
                    in1 = gpk[:, None, :].broadcast_to([NP, IC, WIN])
                    tmp3 = tmp[:].rearrange("p (i d) -> p i d", d=WIN)
                    nc.vector.tensor_tensor(out=tmp3, in0=in0, in1=in1,
                                            op=AluOp.add)
                    dst = macc if ht == 0 else macc2
                    nc.vector.tensor_reduce(
                        out=dst[:, i0:i0 + IC], in_=tmp3,
                        axis=mybir.AxisListType.X, op=AluOp.min,
                    )
                if ht > 0:
                    nc.vector.tensor_tensor(out=macc[:], in0=macc[:],
                                            in1=macc2[:], op=AluOp.min)

            pt = psump.tile([128, 2 * HP], dt)
            for j in range(NG):
                nc.tensor.matmul(
                    pt[j * IW:(j + 1) * IW, :],
                    macc[j * 2 * HP:(j + 1) * 2 * HP, :],
                    ident[j * 2 * HP:(j + 1) * 2 * HP, :],
                    start=True, stop=True,
                )

            cm_ev = bass.AP(tensor=cm_ap.tensor, offset=cm_ap.offset,
                            ap=[list(cm_ap.ap[0]), [2, 2]])
            pt_ap = pt[:]
            pt3 = bass.AP(tensor=pt_ap.tensor, offset=pt_ap.offset,
                          ap=[list(pt_ap.ap[0]), [HP, 2], [1, HP]])
            nc.vector.tensor_reduce(out=cm_ev, in_=pt3,
                                    axis=mybir.AxisListType.X,
                                    op=AluOp.min, negate=True)

            for ih in range(2):
                outt = work.tile([128, _W], dt, tag="outt")
                src = bass.AP(tensor=cm_ap.tensor,
                              offset=cm_ap.offset + 2 * ih,
                              ap=[list(cm_ap.ap[0]), [0, _W // 2], [1, 2]])
                outt_ap = outt[:]
                dst = bass.AP(tensor=outt_ap.tensor, offset=outt_ap.offset,
                              ap=[list(outt_ap.ap[0]), [2, _W // 2], [1, 2]])
                nc.vector.tensor_copy(dst, src)
                eng = nc.sync if ih == 0 else nc.scalar
                eng.dma_start(out=out_ext[ih * 128:(ih + 1) * 128, :],
                              in_=outt[:])

    nc.compile()
    return nc


_USE_RAW = True


def _get_bass(R):
    key = ("fast", R) if R <= _R_FAST_MAX else ("slow", R)
    if key not in _KERNEL_CACHE:
        if key[0] == "fast":
            builder = _build_bass_raw if _USE_RAW else _build_bass_fast
        else:
            builder = _build_bass
        _KERNEL_CACHE[key] = builder(R)
    return _KERNEL_CACHE[key]


def _modd_vec():
    ii = np.arange(_H)
    return np.sqrt(
        np.float32(255.0) ** 2
        + np.maximum(ii, 255 - ii).astype(np.float32) ** 2
    ).astype(np.float32)


def kernel(feature_map, feature_size=None, **_unused):
    from concourse.bass_utils import run_bass_kernel_spmd

    f = np.ascontiguousarray(np.asarray(feature_map, dtype=np.float32))
    assert f.shape == (_B, 1, _H, _W), f.shape

    fmax = float(f.max())
    fmin = float(f.min())
    R = int(np.ceil(fmax - fmin)) + 1
    R = max(2, min(R, _H))

    modd = _modd_vec()
    nc = _get_bass(R)

    if R <= _R_FAST_MAX:
        WIN = 2 * R - 1
        hh = np.arange(R, dtype=np.float32)
        dd = np.arange(-(R - 1), R, dtype=np.float32)
        gtab = np.sqrt(hh[:, None] ** 2 + dd[None, :] ** 2).astype(np.float32)
        gtab_row = gtab.reshape(1, R * WIN)
        moddt = np.ascontiguousarray(modd.reshape(2, 128).T)
        in_maps = [{"pk": _pack_fast(f[b, 0], R, gtab_row), "moddt": moddt}
                   for b in range(_B)]
    else:
        G, HP, NHT, NG, IW, WIN, PW, W2, IC = _params(R)
        hh = np.arange(NHT * HP, dtype=np.float32)
        dd = np.arange(-(R - 1), R, dtype=np.float32)
        gtab = np.sqrt(hh[:, None] ** 2 + dd[None, :] ** 2).astype(np.float32)
        gtab[R:, :] = 0.0
        gdup = np.concatenate([np.tile(gtab[t * HP:(t + 1) * HP], (G, 1))
                               for t in range(NHT)], axis=0)
        moddt = np.ascontiguousarray(modd.reshape(2, 128).T)
        ident = np.ascontiguousarray(
            np.tile(np.eye(2 * HP, dtype=np.float32), (NG, 1)))
        in_maps = []
        for b in range(_B):
            fw = np.full((NHT * HP, W2), _PAD, np.float32)
            fw[:R, R - 1:R - 1 + _W] = f[b, 0, :R, :]
            fpk = np.empty((NHT, 128, PW + WIN), np.float32)
            for j in range(NG):
                for ih in range(2):
                    ib = ih * NG + j
                    p0 = j * 2 * HP + ih * HP
                    for t in range(NHT):
                        fpk[t, p0:p0 + HP, :PW] = \
                            fw[t * HP:(t + 1) * HP, ib * IW:ib * IW + PW]
            fpk[:, :, PW:] = gdup.reshape(NHT, 128, WIN)
            fpk = np.ascontiguousarray(fpk.reshape(NHT * 128, PW + WIN))
            in_maps.append({"fwin": fpk, "moddt": moddt, "ident": ident})

    res = run_bass_kernel_spmd(nc, in_maps, list(range(_N_CORES)))
    out = np.stack([res.results[b]["out"] for b in range(_B)])[:, None]
    return np.ascontiguousarray(out.astype(np.float32))


# revision 26
# speedup vs baseline: 1.1394x; 1.0224x over previous
"""Trainium2 Bass kernel for DistanceTransformLayer2.

Reference semantics (B=8, C=1, H=W=256):
    D_i[h,w] = sqrt(h^2 + (i-w)^2)
    out[b,c,i,j] = max_{h,w} -(D_i[h,w] + f[b,c,h,w])   for even j
    out[b,c,i,j] = max_{h,w} D_i[h,w]                   for odd  j
                 = sqrt(255^2 + max(i,255-i)^2)         (input-independent)

Key algebraic facts used:
  * D_i[h,w] depends only on (h, i-w): D_i[h,w] = g[h,d] with
    d = (i-w)+(R-1) and g[h,d] = sqrt(h^2+(d-(R-1))^2) >= max(h,|d-(R-1)|).
  * Window pruning bound (exact, data-dependent radius R chosen on host):
    since (h=0,w=i) is in the window, the window min of (D+f) is <= fmax.
    Every point outside {h<R, |i-w|<=R-1} has D >= R, so its value is
    >= R + fmin.  Hence for any R >= fmax - fmin the window min equals
    the global min EXACTLY.  R = ceil(fmax-fmin)+1 (~11 for N(0,1)).

Sharding: data-parallel over batch B — core b computes batch b.

Device layout per core (fast path, R <= 88; raw-bass, no TileContext):
  i sits on PARTITIONS (i_lo = i & 127, two ih halves), and the whole
  (h,d) window sits on the FREE axis, im2col-style, packed by the host
  in bf16: pk[i_lo, (1+ih)*RW + h*WIN+d] = f[h, i-(R-1)+d] (PAD
  outside), with the g table in cols [0,RW).  Two HWDGE input DMAs
  (sync ring: [g2|fa], scalar ring: [fb]) run in parallel; the odd
  column constants land in a small fp32 cm tile via a third tiny DMA.
  ONE DVE tensor_tensor add (g broadcast over the ih dim, stride-0 AP)
  and ONE strided-output tensor_reduce(min, negate) produce
      cm[:, ih] = max_{h,d} -(f + g)
  for both halves.  Two broadcast copies (DVE + Activation, in
  parallel) interleave (even=acc, odd=const) into the fp32 output
  tile, and two HWDGE DMAs (sync + scalar) store the row halves.
  (tensor_tensor_reduce would fuse add+reduce in one pass, but its ISA
  encoding wedges real HW on this runtime, and descriptor-count tricks
  don't help: DMA gen is ~0.7us fixed per dma_start.)
"""

import numpy as np

_H = 256
_W = 256
_B = 8
_N_CORES = 8
_PAD = np.float32(1.0e30)
_R_FAST_MAX = 88

_KERNEL_CACHE = {}


def _build_bass_fast(R):
    import concourse.bacc as bacc
    import concourse.bass as bass
    import concourse.mybir as mybir
    from concourse.tile import TileContext

    WIN = 2 * R - 1
    RW = R * WIN

    nc = bacc.Bacc("TRN2", target_bir_lowering=False, debug=False,
                   num_devices=_N_CORES)
    dt = mybir.dt.float32
    bf = mybir.dt.bfloat16
    # pk columns (bf16): [0,RW) g table | [RW,2RW) ih0 win | [2RW,3RW) ih1
    pk_in = nc.dram_tensor("pk", [128, 3 * RW], bf,
                           kind="ExternalInput").ap()
    moddt_in = nc.dram_tensor("moddt", [128, 2], dt,
                              kind="ExternalInput").ap()
    out_ext = nc.dram_tensor("out", [_H, _W], dt, kind="ExternalOutput").ap()

    AluOp = mybir.AluOpType

    with TileContext(nc) as tc:
        with (
            tc.tile_pool(name="work", bufs=1) as work,
        ):
            gfab = work.tile([128, 3 * RW], bf, tag="gfab")
            scratch = work.tile([128, 2 * RW], bf, tag="scratch")
            cm = work.tile([128, 4], dt, tag="cm")
            outt = work.tile([128, 2 * _W], dt, tag="outt")

            # [g2|fa] on the sync HWDGE ring, [fb] on the scalar ring;
            # the tiny odd-column consts follow on the scalar ring
            nc.sync.dma_start(out=gfab[:, 0:2 * RW], in_=pk_in[:, 0:2 * RW])
            nc.scalar.dma_start(out=gfab[:, 2 * RW:3 * RW],
                                in_=pk_in[:, 2 * RW:3 * RW])
            cm_ap = cm[:]
            nc.scalar.dma_start(out=cm[:, 2:4], in_=moddt_in[:])

            # acc[i] = -min_{h,d} (f + g), both ih halves in one ADD+MIN
            # (tensor_tensor_reduce would fuse these but its ISA encoding
            # wedges the HW on this runtime)
            g_ap = gfab[:]
            in1 = bass.AP(tensor=g_ap.tensor, offset=g_ap.offset,
                          ap=[list(g_ap.ap[0]), [0, 2], [1, RW]])
            sc3 = scratch[:].rearrange("p (i x) -> p i x", x=RW)
            nc.vector.tensor_tensor(
                out=sc3, in0=gfab[:, RW:3 * RW].rearrange(
                    "p (i x) -> p i x", x=RW),
                in1=in1, op=AluOp.add)
            nc.vector.tensor_reduce(
                out=cm[:, 0:2], in_=sc3, axis=mybir.AxisListType.X,
                op=AluOp.min, negate=True)

            # interleave (even=acc col ih, odd=const col 2+ih) into outt;
            # ih0 on DVE (follows the reduce in-engine), ih1 on Activation
            # so the two copies and the two out-DMA gens overlap
            o_ap = outt[:]
            for ih in range(2):
                src = bass.AP(tensor=cm_ap.tensor,
                              offset=cm_ap.offset + ih,
                              ap=[list(cm_ap.ap[0]), [0, _W // 2], [2, 2]])
                dst = bass.AP(tensor=o_ap.tensor,
                              offset=o_ap.offset + ih * _W,
                              ap=[list(o_ap.ap[0]), [2, _W // 2], [1, 2]])
                if ih == 0:
                    nc.vector.tensor_copy(dst, src)
                else:
                    nc.scalar.copy(dst, src)

            nc.sync.dma_start(out=out_ext[0:128, :], in_=outt[:, 0:_W])
            nc.scalar.dma_start(out=out_ext[128:256, :],
                                in_=outt[:, _W:2 * _W])

    nc.compile()
    return nc


def _pack_fast(f_b, R, gtab_row):
    """Host-side im2col pack for one batch. f_b: [H, W] fp32 -> bf16."""
    import ml_dtypes
    bf16 = ml_dtypes.bfloat16
    WIN = 2 * R - 1
    RW = R * WIN
    W2 = _W + 2 * (R - 1)
    fw = np.full((R, W2), _PAD, np.float32).astype(bf16)
    fw[:, R - 1:R - 1 + _W] = f_b[:R, :].astype(bf16)
    s0, s1 = fw.strides
    win = np.lib.stride_tricks.as_strided(
        fw, shape=(_H, R, WIN), strides=(s1, s0, s1))
    win2 = win.reshape(_H, RW)
    pk = np.empty((128, 3 * RW), bf16)
    pk[:, 0:RW] = gtab_row.astype(bf16)
    # partition p computes output rows i = p (ih=0) and p+128 (ih=1)
    pk[:, RW:2 * RW] = win2[0:128]
    pk[:, 2 * RW:3 * RW] = win2[128:256]
    return pk


def _build_bass_raw(R):
    """Raw-bass variant of the fast path: same dataflow, manual semaphores,
    no TileContext (skips the tile scheduling prologue/epilogue)."""
    import concourse.bacc as bacc
    import concourse.bass as bass
    import concourse.mybir as mybir

    WIN = 2 * R - 1
    RW = R * WIN

    nc = bacc.Bacc("TRN2", target_bir_lowering=False, debug=False,
                   num_devices=_N_CORES)
    dt = mybir.dt.float32
    bf = mybir.dt.bfloat16
    AluOp = mybir.AluOpType
    pk_in = nc.dram_tensor("pk", [128, 3 * RW], bf,
                           kind="ExternalInput").ap()
    moddt_in = nc.dram_tensor("moddt", [128, 2], dt,
                              kind="ExternalInput").ap()
    out_ext = nc.dram_tensor("out", [_H, _W], dt, kind="ExternalOutput").ap()

    gfab = nc.alloc_sbuf_tensor("gfab", [128, 3 * RW], bf).ap()
    scratch = nc.alloc_sbuf_tensor("scratch", [128, 2 * RW], bf).ap()
    cm = nc.alloc_sbuf_tensor("cm", [128, 4], dt).ap()
    outt = nc.alloc_sbuf_tensor("outt", [128, 2 * _W], dt).ap()

    s_in = nc.alloc_semaphore("s_in")
    s_modd = nc.alloc_semaphore("s_modd")
    s_add = nc.alloc_semaphore("s_add")
    s_min = nc.alloc_semaphore("s_min")
    s_c0 = nc.alloc_semaphore("s_c0")
    s_c1 = nc.alloc_semaphore("s_c1")
    s_out = nc.alloc_semaphore("s_out")

    def cp_aps(ih):
        src = bass.AP(tensor=cm.tensor, offset=cm.offset + ih,
                      ap=[list(cm.ap[0]), [0, _W // 2], [2, 2]])
        dst = bass.AP(tensor=outt.tensor, offset=outt.offset + ih * _W,
                      ap=[list(outt.ap[0]), [2, _W // 2], [1, 2]])
        return src, dst

    with nc.Block("dt2") as blk:
        @blk.sync
        def _(sync):
            sync.dma_start(out=gfab[:, 0:2 * RW],
                           in_=pk_in[:, 0:2 * RW]).then_inc(s_in, 16)
            sync.wait_ge(s_c0, 1)
            sync.dma_start(out=out_ext[0:128, :],
                           in_=outt[:, 0:_W]).then_inc(s_out, 16)
            sync.wait_ge(s_out, 32)

        @blk.scalar
        def _(scalar):
            scalar.dma_start(out=gfab[:, 2 * RW:3 * RW],
                             in_=pk_in[:, 2 * RW:3 * RW]).then_inc(s_in, 16)
            scalar.dma_start(out=cm[:, 2:4],
                             in_=moddt_in[:]).then_inc(s_modd, 16)
            scalar.wait_ge(s_modd, 16)
            scalar.wait_ge(s_min, 1)
            src, dst = cp_aps(1)
            scalar.copy(dst, src).then_inc(s_c1, 1)
            scalar.wait_ge(s_c1, 1)
            scalar.dma_start(out=out_ext[128:256, :],
                             in_=outt[:, _W:2 * _W]).then_inc(s_out, 16)

        @blk.vector
        def _(vector):
            vector.wait_ge(s_in, 32)
            in1 = bass.AP(tensor=gfab.tensor, offset=gfab.offset,
                          ap=[list(gfab.ap[0]), [0, 2], [1, RW]])
            sc3 = scratch.rearrange("p (i x) -> p i x", x=RW)
            vector.tensor_tensor(
                out=sc3,
                in0=gfab[:, RW:3 * RW].rearrange("p (i x) -> p i x", x=RW),
                in1=in1, op=AluOp.add).then_inc(s_add, 1)
            vector.wait_ge(s_modd, 16)
            vector.wait_ge(s_add, 1)
            vector.tensor_reduce(
                out=cm[:, 0:2], in_=sc3, axis=mybir.AxisListType.X,
                op=AluOp.min, negate=True).then_inc(s_min, 1)
            vector.wait_ge(s_min, 1)
            src, dst = cp_aps(0)
            vector.tensor_copy(dst, src).then_inc(s_c0, 1)

    nc.compile()
    return nc


# ---------------------------------------------------------------------------
# Fallback path (original kernel) for large R — partitions pack (j, ih, h),
# vector add+min against a replicated g table, PE-transpose, reduce, copy.
# ---------------------------------------------------------------------------

def _params(R):
    if R <= 32:
        G, HP = 4, 32
    else:
        G, HP = 2, 64
    NHT = -(-R // HP)          # h tiles (1 unless R > 64)
    NG = G // 2                # transpose chunks
    IW = 256 // G              # i width per block
    WIN = 2 * R - 1
    PW = IW + 2 * (R - 1)      # fpk free width per block
    W2 = 256 + 2 * (R - 1)     # host fwin width
    IC = IW
    while IC > 1 and IC * WIN > 16384:
        IC //= 2
    return G, HP, NHT, NG, IW, WIN, PW, W2, IC


def _build_bass(R):
    import concourse.bacc as bacc
    import concourse.bass as bass
    import concourse.mybir as mybir
    from concourse.tile import TileContext

    G, HP, NHT, NG, IW, WIN, PW, W2, IC = _params(R)
    NP = G * HP                # partitions in use (<= 128)
    NIC = IW // IC

    nc = bacc.Bacc("TRN2", target_bir_lowering=False, debug=False,
                   num_devices=_N_CORES)
    dt = mybir.dt.float32
    fwin_in = nc.dram_tensor("fwin", [NHT * 128, PW + WIN], dt,
                             kind="ExternalInput").ap()
    moddt_in = nc.dram_tensor("moddt", [128, 2], dt,
                              kind="ExternalInput").ap()
    ident_in = nc.dram_tensor("ident", [NG * 2 * HP, 2 * HP], dt,
                              kind="ExternalInput").ap()
    out_ext = nc.dram_tensor("out", [_H, _W], dt, kind="ExternalOutput").ap()

    AluOp = mybir.AluOpType

    with TileContext(nc) as tc:
        with (
            tc.tile_pool(name="consts", bufs=1) as consts,
            tc.tile_pool(name="work", bufs=2) as work,
            tc.tile_pool(name="acc", bufs=1) as accp,
            tc.tile_pool(name="psum", bufs=1, space="PSUM") as psump,
        ):
            ident = consts.tile([NG * 2 * HP, 2 * HP], dt)
            nc.gpsimd.dma_start(out=ident[:], in_=ident_in[:])

            cm = consts.tile([128, 4], dt)
            cm_ap = cm[:]
            modd_dst = bass.AP(tensor=cm_ap.tensor, offset=cm_ap.offset + 1,
                               ap=[list(cm_ap.ap[0]), [2, 2]])
            nc.gpsimd.dma_start(out=modd_dst, in_=moddt_in[:])

            macc = accp.tile([NP, IW], dt)
            macc2 = accp.tile([NP, IW], dt)

            for ht in range(NHT):
                fpk = work.tile([NP, PW + WIN], dt, tag="fpk")
                nc.sync.dma_start(
                    out=fpk[:], in_=fwin_in[ht * 128:(ht + 1) * 128, :])
                gpk = fpk[:, PW:PW + WIN]

                for icc in range(NIC):
                    i0 = icc * IC
                    tmp = work.tile([NP, IC * WIN], dt, tag="tmp")
                    fpk_ap = fpk[:]
                    in0 = bass.AP(
                        tensor=fpk_ap.tensor,
                        offset=fpk_ap.offset + i0,
                        ap=[list(fpk_ap.ap[0]), [1, IC], [1, WIN]],
                    )
                    in1 = gpk[:, None, :].broadcast_to([NP, IC, WIN])
                    tmp3 = tmp[:].rearrange("p (i d) -> p i d", d=WIN)
                    nc.vector.tensor_tensor(out=tmp3, in0=in0, in1=in1,
                                            op=AluOp.add)
                    dst = macc if ht == 0 else macc2
                    nc.vector.tensor_reduce(
                        out=dst[:, i0:i0 + IC], in_=tmp3,
                        axis=mybir.AxisListType.X, op=AluOp.min,
                    )
                if ht > 0:
                    nc.vector.tensor_tensor(out=macc[:], in0=macc[:],
                                            in1=macc2[:], op=AluOp.min)

            pt = psump.tile([128, 2 * HP], dt)
            for j in range(NG):
                nc.tensor.matmul(
                    pt[j * IW:(j + 1) * IW, :],
                    macc[j * 2 * HP:(j + 1) * 2 * HP, :],
                    ident[j * 2 * HP:(j + 1) * 2 * HP, :],
                    start=True, stop=True,
                )

            cm_ev = bass.AP(tensor=cm_ap.tensor, offset=cm_ap.offset,
                            ap=[list(cm_ap.ap[0]), [2, 2]])
            pt_ap = pt[:]
            pt3 = bass.AP(tensor=pt_ap.tensor, offset=pt_ap.offset,
                          ap=[list(pt_ap.ap[0]), [HP, 2], [1, HP]])
            nc.vector.tensor_reduce(out=cm_ev, in_=pt3,
                                    axis=mybir.AxisListType.X,
                                    op=AluOp.min, negate=True)

            for ih in range(2):
                outt = work.tile([128, _W], dt, tag="outt")
                src = bass.AP(tensor=cm_ap.tensor,
                              offset=cm_ap.offset + 2 * ih,
                              ap=[list(cm_ap.ap[0]), [0, _W // 2], [1, 2]])
                outt_ap = outt[:]
                dst = bass.AP(tensor=outt_ap.tensor, offset=outt_ap.offset,
                              ap=[list(outt_ap.ap[0]), [2, _W // 2], [1, 2]])
                nc.vector.tensor_copy(dst, src)
                eng = nc.sync if ih == 0 else nc.scalar
                eng.dma_start(out=out_ext[ih * 128:(ih + 1) * 128, :],
                              in_=outt[:])

    nc.compile()
    return nc


_USE_RAW = True


def _get_bass(R):
    key = ("fast", R) if R <= _R_FAST_MAX else ("slow", R)
    if key not in _KERNEL_CACHE:
        if key[0] == "fast":
            builder = _build_bass_raw if _USE_RAW else _build_bass_fast
        else:
            builder = _build_bass
        _KERNEL_CACHE[key] = builder(R)
    return _KERNEL_CACHE[key]


def _modd_vec():
    ii = np.arange(_H)
    return np.sqrt(
        np.float32(255.0) ** 2
        + np.maximum(ii, 255 - ii).astype(np.float32) ** 2
    ).astype(np.float32)


def kernel(feature_map, feature_size=None, **_unused):
    from concourse.bass_utils import run_bass_kernel_spmd

    f = np.ascontiguousarray(np.asarray(feature_map, dtype=np.float32))
    assert f.shape == (_B, 1, _H, _W), f.shape

    fmax = float(f.max())
    fmin = float(f.min())
    R = int(np.ceil(fmax - fmin)) + 1
    R = max(2, min(R, _H))

    modd = _modd_vec()
    nc = _get_bass(R)

    if R <= _R_FAST_MAX:
        WIN = 2 * R - 1
        hh = np.arange(R, dtype=np.float32)
        dd = np.arange(-(R - 1), R, dtype=np.float32)
        gtab = np.sqrt(hh[:, None] ** 2 + dd[None, :] ** 2).astype(np.float32)
        gtab_row = gtab.reshape(1, R * WIN)
        moddt = np.ascontiguousarray(modd.reshape(2, 128).T)
        in_maps = [{"pk": _pack_fast(f[b, 0], R, gtab_row), "moddt": moddt}
                   for b in range(_B)]
    else:
        G, HP, NHT, NG, IW, WIN, PW, W2, IC = _params(R)
        hh = np.arange(NHT * HP, dtype=np.float32)
        dd = np.arange(-(R - 1), R, dtype=np.float32)
        gtab = np.sqrt(hh[:, None] ** 2 + dd[None, :] ** 2).astype(np.float32)
        gtab[R:, :] = 0.0
        gdup = np.concatenate([np.tile(gtab[t * HP:(t + 1) * HP], (G, 1))
                               for t in range(NHT)], axis=0)
        moddt = np.ascontiguousarray(modd.reshape(2, 128).T)
        ident = np.ascontiguousarray(
            np.tile(np.eye(2 * HP, dtype=np.float32), (NG, 1)))
        in_maps = []
        for b in range(_B):
            fw = np.full((NHT * HP, W2), _PAD, np.float32)
            fw[:R, R - 1:R - 1 + _W] = f[b, 0, :R, :]
            fpk = np.empty((NHT, 128, PW + WIN), np.float32)
            for j in range(NG):
                for ih in range(2):
                    ib = ih * NG + j
                    p0 = j * 2 * HP + ih * HP
                    for t in range(NHT):
                        fpk[t, p0:p0 + HP, :PW] = \
                            fw[t * HP:(t + 1) * HP, ib * IW:ib * IW + PW]
            fpk[:, :, PW:] = gdup.reshape(NHT, 128, WIN)
            fpk = np.ascontiguousarray(fpk.reshape(NHT * 128, PW + WIN))
            in_maps.append({"fwin": fpk, "moddt": moddt, "ident": ident})

    res = run_bass_kernel_spmd(nc, in_maps, list(range(_N_CORES)))
    out = np.stack([res.results[b]["out"] for b in range(_B)])[:, None]
    return np.ascontiguousarray(out.astype(np.float32))


# revision 27
# speedup vs baseline: 1.2025x; 1.0554x over previous
"""Trainium2 Bass kernel for DistanceTransformLayer2.

Reference semantics (B=8, C=1, H=W=256):
    D_i[h,w] = sqrt(h^2 + (i-w)^2)
    out[b,c,i,j] = max_{h,w} -(D_i[h,w] + f[b,c,h,w])   for even j
    out[b,c,i,j] = max_{h,w} D_i[h,w]                   for odd  j
                 = sqrt(255^2 + max(i,255-i)^2)         (input-independent)

Key algebraic facts used:
  * D_i[h,w] depends only on (h, i-w): D_i[h,w] = g[h,d] with
    d = (i-w)+(R-1) and g[h,d] = sqrt(h^2+(d-(R-1))^2) >= max(h,|d-(R-1)|).
  * Window pruning bound (exact, data-dependent radius R chosen on host):
    since (h=0,w=i) is in the window, the window min of (D+f) is <= fmax.
    Every point outside {h<R, |i-w|<=R-1} has D >= R, so its value is
    >= R + fmin.  Hence for any R >= fmax - fmin the window min equals
    the global min EXACTLY.  R = ceil(fmax-fmin)+1 (~11 for N(0,1)).

Sharding: data-parallel over batch B — core b computes batch b.

Device layout per core (fast path, R <= 88; raw-bass, no TileContext):
  i sits on PARTITIONS (i_lo = i & 127, two ih halves), and the whole
  (h,d) window sits on the FREE axis, im2col-style, packed by the host
  in bf16: pk[i_lo, (1+ih)*RW + h*WIN+d] = f[h, i-(R-1)+d] (PAD
  outside), with the g table in cols [0,RW).  Two HWDGE input DMAs
  (sync ring: [g2|fa], scalar ring: [fb]) run in parallel; the odd
  column constants land in a small fp32 cm tile via a third tiny DMA.
  ONE DVE tensor_tensor add (g broadcast over the ih dim, stride-0 AP)
  and ONE strided-output tensor_reduce(min, negate) produce
      cm[:, ih] = max_{h,d} -(f + g)
  for both halves.  Two broadcast copies (DVE + Activation, in
  parallel) interleave (even=acc, odd=const) into the fp32 output
  tile, and two HWDGE DMAs (sync + scalar) store the row halves.
  (tensor_tensor_reduce would fuse add+reduce in one pass, but its ISA
  encoding wedges real HW on this runtime, and descriptor-count tricks
  don't help: DMA gen is ~0.7us fixed per dma_start.)
"""

import numpy as np

_H = 256
_W = 256
_B = 8
_N_CORES = 8
_PAD = np.float32(1.0e30)
_R_FAST_MAX = 88

_KERNEL_CACHE = {}


def _build_bass_fast(R):
    import concourse.bacc as bacc
    import concourse.bass as bass
    import concourse.mybir as mybir
    from concourse.tile import TileContext

    WIN = 2 * R - 1
    RW = R * WIN

    nc = bacc.Bacc("TRN2", target_bir_lowering=False, debug=False,
                   num_devices=_N_CORES)
    dt = mybir.dt.float32
    bf = mybir.dt.bfloat16
    # pk columns (bf16): [0,RW) g table | [RW,2RW) ih0 win | [2RW,3RW) ih1
    pk_in = nc.dram_tensor("pk", [128, 3 * RW], bf,
                           kind="ExternalInput").ap()
    moddt_in = nc.dram_tensor("moddt", [128, 2], dt,
                              kind="ExternalInput").ap()
    out_ext = nc.dram_tensor("out", [_H, _W], dt, kind="ExternalOutput").ap()

    AluOp = mybir.AluOpType

    with TileContext(nc) as tc:
        with (
            tc.tile_pool(name="work", bufs=1) as work,
        ):
            gfab = work.tile([128, 3 * RW], bf, tag="gfab")
            scratch = work.tile([128, 2 * RW], bf, tag="scratch")
            cm = work.tile([128, 4], dt, tag="cm")
            outt = work.tile([128, 2 * _W], dt, tag="outt")

            # [g2|fa] on the sync HWDGE ring, [fb] on the scalar ring;
            # the tiny odd-column consts follow on the scalar ring
            nc.sync.dma_start(out=gfab[:, 0:2 * RW], in_=pk_in[:, 0:2 * RW])
            nc.scalar.dma_start(out=gfab[:, 2 * RW:3 * RW],
                                in_=pk_in[:, 2 * RW:3 * RW])
            cm_ap = cm[:]
            nc.scalar.dma_start(out=cm[:, 2:4], in_=moddt_in[:])

            # acc[i] = -min_{h,d} (f + g), both ih halves in one ADD+MIN
            # (tensor_tensor_reduce would fuse these but its ISA encoding
            # wedges the HW on this runtime)
            g_ap = gfab[:]
            in1 = bass.AP(tensor=g_ap.tensor, offset=g_ap.offset,
                          ap=[list(g_ap.ap[0]), [0, 2], [1, RW]])
            sc3 = scratch[:].rearrange("p (i x) -> p i x", x=RW)
            nc.vector.tensor_tensor(
                out=sc3, in0=gfab[:, RW:3 * RW].rearrange(
                    "p (i x) -> p i x", x=RW),
                in1=in1, op=AluOp.add)
            nc.vector.tensor_reduce(
                out=cm[:, 0:2], in_=sc3, axis=mybir.AxisListType.X,
                op=AluOp.min, negate=True)

            # interleave (even=acc col ih, odd=const col 2+ih) into outt;
            # ih0 on DVE (follows the reduce in-engine), ih1 on Activation
            # so the two copies and the two out-DMA gens overlap
            o_ap = outt[:]
            for ih in range(2):
                src = bass.AP(tensor=cm_ap.tensor,
                              offset=cm_ap.offset + ih,
                              ap=[list(cm_ap.ap[0]), [0, _W // 2], [2, 2]])
                dst = bass.AP(tensor=o_ap.tensor,
                              offset=o_ap.offset + ih * _W,
                              ap=[list(o_ap.ap[0]), [2, _W // 2], [1, 2]])
                if ih == 0:
                    nc.vector.tensor_copy(dst, src)
                else:
                    nc.scalar.copy(dst, src)

            nc.sync.dma_start(out=out_ext[0:128, :], in_=outt[:, 0:_W])
            nc.scalar.dma_start(out=out_ext[128:256, :],
                                in_=outt[:, _W:2 * _W])

    nc.compile()
    return nc


def _pack_fast(f_b, R, gtab_row):
    """Host-side im2col pack for one batch. f_b: [H, W] fp32 -> bf16."""
    import ml_dtypes
    bf16 = ml_dtypes.bfloat16
    WIN = 2 * R - 1
    RW = R * WIN
    W2 = _W + 2 * (R - 1)
    fw = np.full((R, W2), _PAD, np.float32).astype(bf16)
    fw[:, R - 1:R - 1 + _W] = f_b[:R, :].astype(bf16)
    s0, s1 = fw.strides
    win = np.lib.stride_tricks.as_strided(
        fw, shape=(_H, R, WIN), strides=(s1, s0, s1))
    win2 = win.reshape(_H, RW)
    pk = np.empty((128, 3 * RW), bf16)
    pk[:, 0:RW] = gtab_row.astype(bf16)
    # partition p computes output rows i = p (ih=0) and p+128 (ih=1)
    pk[:, RW:2 * RW] = win2[0:128]
    pk[:, 2 * RW:3 * RW] = win2[128:256]
    return pk


def _build_bass_raw(R):
    """Raw-bass variant of the fast path: same dataflow, manual semaphores,
    no TileContext (skips the tile scheduling prologue/epilogue)."""
    import concourse.bacc as bacc
    import concourse.bass as bass
    import concourse.mybir as mybir

    WIN = 2 * R - 1
    RW = R * WIN

    nc = bacc.Bacc("TRN2", target_bir_lowering=False, debug=False,
                   num_devices=_N_CORES)
    dt = mybir.dt.float32
    bf = mybir.dt.bfloat16
    AluOp = mybir.AluOpType
    pk_in = nc.dram_tensor("pk", [128, 3 * RW], bf,
                           kind="ExternalInput").ap()
    moddt_in = nc.dram_tensor("moddt", [128, 2], dt,
                              kind="ExternalInput").ap()
    out_ext = nc.dram_tensor("out", [_H, _W], dt, kind="ExternalOutput").ap()

    gfab = nc.alloc_sbuf_tensor("gfab", [128, 3 * RW], bf).ap()
    scratch = nc.alloc_sbuf_tensor("scratch", [128, 2 * RW], bf).ap()
    cm = nc.alloc_sbuf_tensor("cm", [128, 4], dt).ap()
    outt = nc.alloc_sbuf_tensor("outt", [128, 2 * _W], dt).ap()

    s_in = nc.alloc_semaphore("s_in")
    s_modd = nc.alloc_semaphore("s_modd")
    s_add = nc.alloc_semaphore("s_add")
    s_min = nc.alloc_semaphore("s_min")
    s_c0 = nc.alloc_semaphore("s_c0")
    s_c1 = nc.alloc_semaphore("s_c1")
    s_out = nc.alloc_semaphore("s_out")

    def cp_aps(ih):
        src = bass.AP(tensor=cm.tensor, offset=cm.offset + ih,
                      ap=[list(cm.ap[0]), [0, _W // 2], [2, 2]])
        dst = bass.AP(tensor=outt.tensor, offset=outt.offset + ih * _W,
                      ap=[list(outt.ap[0]), [2, _W // 2], [1, 2]])
        return src, dst

    with nc.Block("dt2") as blk:
        @blk.sync
        def _(sync):
            sync.dma_start(out=gfab[:, 0:2 * RW],
                           in_=pk_in[:, 0:2 * RW]).then_inc(s_in, 16)
            sync.wait_ge(s_c0, 1)
            sync.dma_start(out=out_ext[0:128, :],
                           in_=outt[:, 0:_W]).then_inc(s_out, 16)
            sync.wait_ge(s_out, 32)

        @blk.scalar
        def _(scalar):
            scalar.dma_start(out=gfab[:, 2 * RW:3 * RW],
                             in_=pk_in[:, 2 * RW:3 * RW]).then_inc(s_in, 16)
            scalar.dma_start(out=cm[:, 2:4],
                             in_=moddt_in[:]).then_inc(s_modd, 16)
            scalar.wait_ge(s_modd, 16)
            scalar.wait_ge(s_min, 1)
            src, dst = cp_aps(1)
            scalar.copy(dst, src).then_inc(s_c1, 1)
            scalar.wait_ge(s_c1, 1)
            scalar.dma_start(out=out_ext[128:256, :],
                             in_=outt[:, _W:2 * _W]).then_inc(s_out, 16)

        @blk.vector
        def _(vector):
            vector.wait_ge(s_in, 32)
            in1 = bass.AP(tensor=gfab.tensor, offset=gfab.offset,
                          ap=[list(gfab.ap[0]), [0, 2], [1, RW]])
            sc3 = scratch.rearrange("p (i x) -> p i x", x=RW)
            vector.tensor_tensor(
                out=sc3,
                in0=gfab[:, RW:3 * RW].rearrange("p (i x) -> p i x", x=RW),
                in1=in1, op=AluOp.add).then_inc(s_add, 1)
            vector.wait_ge(s_modd, 16)
            vector.wait_ge(s_add, 1)
            vector.tensor_reduce(
                out=cm[:, 0:2], in_=sc3, axis=mybir.AxisListType.X,
                op=AluOp.min, negate=True).then_inc(s_min, 1)
            vector.wait_ge(s_min, 1)
            src, dst = cp_aps(0)
            vector.tensor_copy(dst, src).then_inc(s_c0, 1)

    nc.compile()
    return nc


# ---------------------------------------------------------------------------
# Fallback path (original kernel) for large R — partitions pack (j, ih, h),
# vector add+min against a replicated g table, PE-transpose, reduce, copy.
# ---------------------------------------------------------------------------

def _params(R):
    if R <= 32:
        G, HP = 4, 32
    else:
        G, HP = 2, 64
    NHT = -(-R // HP)          # h tiles (1 unless R > 64)
    NG = G // 2                # transpose chunks
    IW = 256 // G              # i width per block
    WIN = 2 * R - 1
    PW = IW + 2 * (R - 1)      # fpk free width per block
    W2 = 256 + 2 * (R - 1)     # host fwin width
    IC = IW
    while IC > 1 and IC * WIN > 16384:
        IC //= 2
    return G, HP, NHT, NG, IW, WIN, PW, W2, IC


def _build_bass(R):
    import concourse.bacc as bacc
    import concourse.bass as bass
    import concourse.mybir as mybir
    from concourse.tile import TileContext

    G, HP, NHT, NG, IW, WIN, PW, W2, IC = _params(R)
    NP = G * HP                # partitions in use (<= 128)
    NIC = IW // IC

    nc = bacc.Bacc("TRN2", target_bir_lowering=False, debug=False,
                   num_devices=_N_CORES)
    dt = mybir.dt.float32
    fwin_in = nc.dram_tensor("fwin", [NHT * 128, PW + WIN], dt,
                             kind="ExternalInput").ap()
    moddt_in = nc.dram_tensor("moddt", [128, 2], dt,
                              kind="ExternalInput").ap()
    ident_in = nc.dram_tensor("ident", [NG * 2 * HP, 2 * HP], dt,
                              kind="ExternalInput").ap()
    out_ext = nc.dram_tensor("out", [_H, _W], dt, kind="ExternalOutput").ap()

    AluOp = mybir.AluOpType

    with TileContext(nc) as tc:
        with (
            tc.tile_pool(name="consts", bufs=1) as consts,
            tc.tile_pool(name="work", bufs=2) as work,
            tc.tile_pool(name="acc", bufs=1) as accp,
            tc.tile_pool(name="psum", bufs=1, space="PSUM") as psump,
        ):
            ident = consts.tile([NG * 2 * HP, 2 * HP], dt)
            nc.gpsimd.dma_start(out=ident[:], in_=ident_in[:])

            cm = consts.tile([128, 4], dt)
            cm_ap = cm[:]
            modd_dst = bass.AP(tensor=cm_ap.tensor, offset=cm_ap.offset + 1,
                               ap=[list(cm_ap.ap[0]), [2, 2]])
            nc.gpsimd.dma_start(out=modd_dst, in_=moddt_in[:])

            macc = accp.tile([NP, IW], dt)
            macc2 = accp.tile([NP, IW], dt)

            for ht in range(NHT):
                fpk = work.tile([NP, PW + WIN], dt, tag="fpk")
                nc.sync.dma_start(
                    out=fpk[:], in_=fwin_in[ht * 128:(ht + 1) * 128, :])
                gpk = fpk[:, PW:PW + WIN]

                for icc in range(NIC):
                    i0 = icc * IC
                    tmp = work.tile([NP, IC * WIN], dt, tag="tmp")
                    fpk_ap = fpk[:]
                    in0 = bass.AP(
                        tensor=fpk_ap.tensor,
                        offset=fpk_ap.offset + i0,
                        ap=[list(fpk_ap.ap[0]), [1, IC], [1, WIN]],
                    )
                    in1 = gpk[:, None, :].broadcast_to([NP, IC, WIN])
                    tmp3 = tmp[:].rearrange("p (i d) -> p i d", d=WIN)
                    nc.vector.tensor_tensor(out=tmp3, in0=in0, in1=in1,
                                            op=AluOp.add)
                    dst = macc if ht == 0 else macc2
                    nc.vector.tensor_reduce(
                        out=dst[:, i0:i0 + IC], in_=tmp3,
                        axis=mybir.AxisListType.X, op=AluOp.min,
                    )
                if ht > 0:
                    nc.vector.tensor_tensor(out=macc[:], in0=macc[:],
                                            in1=macc2[:], op=AluOp.min)

            pt = psump.tile([128, 2 * HP], dt)
            for j in range(NG):
                nc.tensor.matmul(
                    pt[j * IW:(j + 1) * IW, :],
                    macc[j * 2 * HP:(j + 1) * 2 * HP, :],
                    ident[j * 2 * HP:(j + 1) * 2 * HP, :],
                    start=True, stop=True,
                )

            cm_ev = bass.AP(tensor=cm_ap.tensor, offset=cm_ap.offset,
                            ap=[list(cm_ap.ap[0]), [2, 2]])
            pt_ap = pt[:]
            pt3 = bass.AP(tensor=pt_ap.tensor, offset=pt_ap.offset,
                          ap=[list(pt_ap.ap[0]), [HP, 2], [1, HP]])
            nc.vector.tensor_reduce(out=cm_ev, in_=pt3,
                                    axis=mybir.AxisListType.X,
                                    op=AluOp.min, negate=True)

            for ih in range(2):
                outt = work.tile([128, _W], dt, tag="outt")
                src = bass.AP(tensor=cm_ap.tensor,
                              offset=cm_ap.offset + 2 * ih,
                              ap=[list(cm_ap.ap[0]), [0, _W // 2], [1, 2]])
                outt_ap = outt[:]
                dst = bass.AP(tensor=outt_ap.tensor, offset=outt_ap.offset,
                              ap=[list(outt_ap.ap[0]), [2, _W // 2], [1, 2]])
                nc.vector.tensor_copy(dst, src)
                eng = nc.sync if ih == 0 else nc.scalar
                eng.dma_start(out=out_ext[ih * 128:(ih + 1) * 128, :],
                              in_=outt[:])

    nc.compile()
    return nc


_USE_RAW = True


def _get_bass(R):
    key = ("fast", R) if R <= _R_FAST_MAX else ("slow", R)
    if key not in _KERNEL_CACHE:
        if key[0] == "fast":
            builder = _build_bass_raw if _USE_RAW else _build_bass_fast
        else:
            builder = _build_bass
        _KERNEL_CACHE[key] = builder(R)
    return _KERNEL_CACHE[key]


def _modd_vec():
    ii = np.arange(_H)
    return np.sqrt(
        np.float32(255.0) ** 2
        + np.maximum(ii, 255 - ii).astype(np.float32) ** 2
    ).astype(np.float32)


def kernel(feature_map, feature_size=None, **_unused):
    from concourse.bass_utils import run_bass_kernel_spmd

    f = np.ascontiguousarray(np.asarray(feature_map, dtype=np.float32))
    assert f.shape == (_B, 1, _H, _W), f.shape

    # Exact pruning radius. The window min for (b,i) is upper-bounded by
    # U[b,i] = min_{|d|<=r0} (f[b,0,0,i+d] + |d|)  (witnesses at h=0, and
    # |d| <= D there), while every point with max(h,|i-w|) >= R is
    # >= R + fmin.  So R >= ceil(max U - fmin) + 1 keeps the windowed min
    # EXACTLY equal to the global min (+1 also covers bf16 rounding).
    fmin = float(f.min())
    f0 = f[:, 0, 0, :]

    def _ubound(r0):
        U = np.full_like(f0, np.inf)
        for d in range(-r0, r0 + 1):
            lo, hi = max(0, -d), _W - max(0, d)
            U[:, lo:hi] = np.minimum(U[:, lo:hi],
                                     f0[:, lo + d:hi + d] + abs(d))
        return float(U.max())

    r0 = 4
    R = int(np.ceil(_ubound(r0) - fmin)) + 1
    if R - 1 < r0:
        # U's witnesses must lie inside the window: redo with a valid r0
        R = int(np.ceil(_ubound(max(0, R - 1)) - fmin)) + 1
    R = max(2, min(R, _H))

    modd = _modd_vec()
    nc = _get_bass(R)

    if R <= _R_FAST_MAX:
        WIN = 2 * R - 1
        hh = np.arange(R, dtype=np.float32)
        dd = np.arange(-(R - 1), R, dtype=np.float32)
        gtab = np.sqrt(hh[:, None] ** 2 + dd[None, :] ** 2).astype(np.float32)
        gtab_row = gtab.reshape(1, R * WIN)
        moddt = np.ascontiguousarray(modd.reshape(2, 128).T)
        in_maps = [{"pk": _pack_fast(f[b, 0], R, gtab_row), "moddt": moddt}
                   for b in range(_B)]
    else:
        G, HP, NHT, NG, IW, WIN, PW, W2, IC = _params(R)
        hh = np.arange(NHT * HP, dtype=np.float32)
        dd = np.arange(-(R - 1), R, dtype=np.float32)
        gtab = np.sqrt(hh[:, None] ** 2 + dd[None, :] ** 2).astype(np.float32)
        gtab[R:, :] = 0.0
        gdup = np.concatenate([np.tile(gtab[t * HP:(t + 1) * HP], (G, 1))
                               for t in range(NHT)], axis=0)
        moddt = np.ascontiguousarray(modd.reshape(2, 128).T)
        ident = np.ascontiguousarray(
            np.tile(np.eye(2 * HP, dtype=np.float32), (NG, 1)))
        in_maps = []
        for b in range(_B):
            fw = np.full((NHT * HP, W2), _PAD, np.float32)
            fw[:R, R - 1:R - 1 + _W] = f[b, 0, :R, :]
            fpk = np.empty((NHT, 128, PW + WIN), np.float32)
            for j in range(NG):
                for ih in range(2):
                    ib = ih * NG + j
                    p0 = j * 2 * HP + ih * HP
                    for t in range(NHT):
                        fpk[t, p0:p0 + HP, :PW] = \
                            fw[t * HP:(t + 1) * HP, ib * IW:ib * IW + PW]
            fpk[:, :, PW:] = gdup.reshape(NHT, 128, WIN)
            fpk = np.ascontiguousarray(fpk.reshape(NHT * 128, PW + WIN))
            in_maps.append({"fwin": fpk, "moddt": moddt, "ident": ident})

    res = run_bass_kernel_spmd(nc, in_maps, list(range(_N_CORES)))
    out = np.stack([res.results[b]["out"] for b in range(_B)])[:, None]
    return np.ascontiguousarray(out.astype(np.float32))


# revision 28
# speedup vs baseline: 1.2239x; 1.0178x over previous
"""Trainium2 Bass kernel for DistanceTransformLayer2.

Reference semantics (B=8, C=1, H=W=256):
    D_i[h,w] = sqrt(h^2 + (i-w)^2)
    out[b,c,i,j] = max_{h,w} -(D_i[h,w] + f[b,c,h,w])   for even j
    out[b,c,i,j] = max_{h,w} D_i[h,w]                   for odd  j
                 = sqrt(255^2 + max(i,255-i)^2)         (input-independent)

Key algebraic facts used:
  * D_i[h,w] depends only on (h, i-w): D_i[h,w] = g[h,d] with
    d = (i-w)+(R-1) and g[h,d] = sqrt(h^2+(d-(R-1))^2) >= max(h,|d-(R-1)|).
  * Window pruning bound (exact, data-dependent radius R chosen on host):
    since (h=0,w=i) is in the window, the window min of (D+f) is <= fmax.
    Every point outside {h<R, |i-w|<=R-1} has D >= R, so its value is
    >= R + fmin.  Hence for any R >= fmax - fmin the window min equals
    the global min EXACTLY.  R = ceil(fmax-fmin)+1 (~11 for N(0,1)).

Sharding: data-parallel over batch B — core b computes batch b.

Device layout per core (fast path, R <= 88; raw-bass, no TileContext):
  i sits on PARTITIONS (i_lo = i & 127, two ih halves), and the whole
  (h,d) window sits on the FREE axis, im2col-style, packed by the host
  in bf16: pk[i_lo, (1+ih)*RW + h*WIN+d] = f[h, i-(R-1)+d] (PAD
  outside), with the g table in cols [0,RW).  Two HWDGE input DMAs
  (sync ring: [g2|fa], scalar ring: [fb]) run in parallel; the odd
  column constants land in a small fp32 cm tile via a third tiny DMA.
  ONE DVE tensor_tensor add (g broadcast over the ih dim, stride-0 AP)
  and ONE strided-output tensor_reduce(min, negate) produce
      cm[:, ih] = max_{h,d} -(f + g)
  for both halves.  Two broadcast copies (DVE + Activation, in
  parallel) interleave (even=acc, odd=const) into the fp32 output
  tile, and two HWDGE DMAs (sync + scalar) store the row halves.
  (tensor_tensor_reduce would fuse add+reduce in one pass, but its ISA
  encoding wedges real HW on this runtime, and descriptor-count tricks
  don't help: DMA gen is ~0.7us fixed per dma_start.)
"""

import numpy as np

_H = 256
_W = 256
_B = 8
_N_CORES = 8
_PAD = np.float32(1.0e30)
_R_FAST_MAX = 88

_KERNEL_CACHE = {}


def _build_bass_fast(R):
    import concourse.bacc as bacc
    import concourse.bass as bass
    import concourse.mybir as mybir
    from concourse.tile import TileContext

    WIN = 2 * R - 1
    RW = R * WIN

    nc = bacc.Bacc("TRN2", target_bir_lowering=False, debug=False,
                   num_devices=_N_CORES)
    dt = mybir.dt.float32
    bf = mybir.dt.bfloat16
    # pk columns (bf16): [0,RW) g table | [RW,2RW) ih0 win | [2RW,3RW) ih1
    pk_in = nc.dram_tensor("pk", [128, 3 * RW], bf,
                           kind="ExternalInput").ap()
    moddt_in = nc.dram_tensor("moddt", [128, 2], dt,
                              kind="ExternalInput").ap()
    out_ext = nc.dram_tensor("out", [_H, _W], dt, kind="ExternalOutput").ap()

    AluOp = mybir.AluOpType

    with TileContext(nc) as tc:
        with (
            tc.tile_pool(name="work", bufs=1) as work,
        ):
            gfab = work.tile([128, 3 * RW], bf, tag="gfab")
            scratch = work.tile([128, 2 * RW], bf, tag="scratch")
            cm = work.tile([128, 4], dt, tag="cm")
            outt = work.tile([128, 2 * _W], dt, tag="outt")

            # [g2|fa] on the sync HWDGE ring, [fb] on the scalar ring;
            # the tiny odd-column consts follow on the scalar ring
            nc.sync.dma_start(out=gfab[:, 0:2 * RW], in_=pk_in[:, 0:2 * RW])
            nc.scalar.dma_start(out=gfab[:, 2 * RW:3 * RW],
                                in_=pk_in[:, 2 * RW:3 * RW])
            cm_ap = cm[:]
            nc.scalar.dma_start(out=cm[:, 2:4], in_=moddt_in[:])

            # acc[i] = -min_{h,d} (f + g), both ih halves in one ADD+MIN
            # (tensor_tensor_reduce would fuse these but its ISA encoding
            # wedges the HW on this runtime)
            g_ap = gfab[:]
            in1 = bass.AP(tensor=g_ap.tensor, offset=g_ap.offset,
                          ap=[list(g_ap.ap[0]), [0, 2], [1, RW]])
            sc3 = scratch[:].rearrange("p (i x) -> p i x", x=RW)
            nc.vector.tensor_tensor(
                out=sc3, in0=gfab[:, RW:3 * RW].rearrange(
                    "p (i x) -> p i x", x=RW),
                in1=in1, op=AluOp.add)
            nc.vector.tensor_reduce(
                out=cm[:, 0:2], in_=sc3, axis=mybir.AxisListType.X,
                op=AluOp.min, negate=True)

            # interleave (even=acc col ih, odd=const col 2+ih) into outt;
            # ih0 on DVE (follows the reduce in-engine), ih1 on Activation
            # so the two copies and the two out-DMA gens overlap
            o_ap = outt[:]
            for ih in range(2):
                src = bass.AP(tensor=cm_ap.tensor,
                              offset=cm_ap.offset + ih,
                              ap=[list(cm_ap.ap[0]), [0, _W // 2], [2, 2]])
                dst = bass.AP(tensor=o_ap.tensor,
                              offset=o_ap.offset + ih * _W,
                              ap=[list(o_ap.ap[0]), [2, _W // 2], [1, 2]])
                if ih == 0:
                    nc.vector.tensor_copy(dst, src)
                else:
                    nc.scalar.copy(dst, src)

            nc.sync.dma_start(out=out_ext[0:128, :], in_=outt[:, 0:_W])
            nc.scalar.dma_start(out=out_ext[128:256, :],
                                in_=outt[:, _W:2 * _W])

    nc.compile()
    return nc


def _pack_fast(f_b, R, gtab_row):
    """Host-side im2col pack for one batch. f_b: [H, W] fp32 -> bf16."""
    import ml_dtypes
    bf16 = ml_dtypes.bfloat16
    WIN = 2 * R - 1
    RW = R * WIN
    W2 = _W + 2 * (R - 1)
    fw = np.full((R, W2), _PAD, np.float32).astype(bf16)
    fw[:, R - 1:R - 1 + _W] = f_b[:R, :].astype(bf16)
    s0, s1 = fw.strides
    win = np.lib.stride_tricks.as_strided(
        fw, shape=(_H, R, WIN), strides=(s1, s0, s1))
    win2 = win.reshape(_H, RW)
    pk = np.empty((128, 3 * RW), bf16)
    pk[:, 0:RW] = gtab_row.astype(bf16)
    # partition p computes output rows i = p (ih=0) and p+128 (ih=1)
    pk[:, RW:2 * RW] = win2[0:128]
    pk[:, 2 * RW:3 * RW] = win2[128:256]
    return pk


def _build_bass_raw(R):
    """Raw-bass variant of the fast path: same dataflow, manual semaphores,
    no TileContext (skips the tile scheduling prologue/epilogue)."""
    import concourse.bacc as bacc
    import concourse.bass as bass
    import concourse.mybir as mybir

    WIN = 2 * R - 1
    RW = R * WIN

    nc = bacc.Bacc("TRN2", target_bir_lowering=False, debug=False,
                   num_devices=_N_CORES)
    dt = mybir.dt.float32
    bf = mybir.dt.bfloat16
    AluOp = mybir.AluOpType
    pk_in = nc.dram_tensor("pk", [128, 3 * RW], bf,
                           kind="ExternalInput").ap()
    moddt_in = nc.dram_tensor("moddt", [128, 2], dt,
                              kind="ExternalInput").ap()
    out_ext = nc.dram_tensor("out", [_H, _W], dt, kind="ExternalOutput").ap()

    gfab = nc.alloc_sbuf_tensor("gfab", [128, 3 * RW], bf).ap()
    scratch = nc.alloc_sbuf_tensor("scratch", [128, 2 * RW], bf).ap()
    cm = nc.alloc_sbuf_tensor("cm", [128, 4], dt).ap()
    outt = nc.alloc_sbuf_tensor("outt", [128, 2 * _W], dt).ap()

    s_in = nc.alloc_semaphore("s_in")
    s_modd = nc.alloc_semaphore("s_modd")
    s_add = nc.alloc_semaphore("s_add")
    s_min = nc.alloc_semaphore("s_min")
    s_c0 = nc.alloc_semaphore("s_c0")
    s_c1 = nc.alloc_semaphore("s_c1")
    s_out = nc.alloc_semaphore("s_out")

    def cp_aps(ih):
        src = bass.AP(tensor=cm.tensor, offset=cm.offset + ih,
                      ap=[list(cm.ap[0]), [0, _W // 2], [2, 2]])
        dst = bass.AP(tensor=outt.tensor, offset=outt.offset + ih * _W,
                      ap=[list(outt.ap[0]), [2, _W // 2], [1, 2]])
        return src, dst

    with nc.Block("dt2") as blk:
        @blk.sync
        def _(sync):
            sync.dma_start(out=gfab[:, 0:2 * RW],
                           in_=pk_in[:, 0:2 * RW]).then_inc(s_in, 16)
            sync.dma_start(out=cm[:, 2:4],
                           in_=moddt_in[:]).then_inc(s_modd, 16)
            sync.wait_ge(s_c0, 1)
            sync.dma_start(out=out_ext[0:128, :],
                           in_=outt[:, 0:_W]).then_inc(s_out, 16)
            sync.wait_ge(s_out, 32)

        @blk.scalar
        def _(scalar):
            scalar.dma_start(out=gfab[:, 2 * RW:3 * RW],
                             in_=pk_in[:, 2 * RW:3 * RW]).then_inc(s_in, 16)
            scalar.wait_ge(s_c1, 1)
            scalar.dma_start(out=out_ext[128:256, :],
                             in_=outt[:, _W:2 * _W]).then_inc(s_out, 16)

        @blk.vector
        def _(vector):
            vector.wait_ge(s_in, 32)
            in1 = bass.AP(tensor=gfab.tensor, offset=gfab.offset,
                          ap=[list(gfab.ap[0]), [0, 2], [1, RW]])
            sc3 = scratch.rearrange("p (i x) -> p i x", x=RW)
            vector.tensor_tensor(
                out=sc3,
                in0=gfab[:, RW:3 * RW].rearrange("p (i x) -> p i x", x=RW),
                in1=in1, op=AluOp.add).then_inc(s_add, 1)
            vector.wait_ge(s_add, 1)
            vector.tensor_reduce(
                out=cm[:, 0:2], in_=sc3, axis=mybir.AxisListType.X,
                op=AluOp.min, negate=True).then_inc(s_min, 1)
            vector.wait_ge(s_min, 1)
            vector.wait_ge(s_modd, 16)
            src, dst = cp_aps(0)
            vector.tensor_copy(dst, src).then_inc(s_c0, 1)
            vector.wait_ge(s_min, 1)
            src, dst = cp_aps(1)
            vector.tensor_copy(dst, src).then_inc(s_c1, 1)

    nc.compile()
    return nc


# ---------------------------------------------------------------------------
# Fallback path (original kernel) for large R — partitions pack (j, ih, h),
# vector add+min against a replicated g table, PE-transpose, reduce, copy.
# ---------------------------------------------------------------------------

def _params(R):
    if R <= 32:
        G, HP = 4, 32
    else:
        G, HP = 2, 64
    NHT = -(-R // HP)          # h tiles (1 unless R > 64)
    NG = G // 2                # transpose chunks
    IW = 256 // G              # i width per block
    WIN = 2 * R - 1
    PW = IW + 2 * (R - 1)      # fpk free width per block
    W2 = 256 + 2 * (R - 1)     # host fwin width
    IC = IW
    while IC > 1 and IC * WIN > 16384:
        IC //= 2
    return G, HP, NHT, NG, IW, WIN, PW, W2, IC


def _build_bass(R):
    import concourse.bacc as bacc
    import concourse.bass as bass
    import concourse.mybir as mybir
    from concourse.tile import TileContext

    G, HP, NHT, NG, IW, WIN, PW, W2, IC = _params(R)
    NP = G * HP                # partitions in use (<= 128)
    NIC = IW // IC

    nc = bacc.Bacc("TRN2", target_bir_lowering=False, debug=False,
                   num_devices=_N_CORES)
    dt = mybir.dt.float32
    fwin_in = nc.dram_tensor("fwin", [NHT * 128, PW + WIN], dt,
                             kind="ExternalInput").ap()
    moddt_in = nc.dram_tensor("moddt", [128, 2], dt,
                              kind="ExternalInput").ap()
    ident_in = nc.dram_tensor("ident", [NG * 2 * HP, 2 * HP], dt,
                              kind="ExternalInput").ap()
    out_ext = nc.dram_tensor("out", [_H, _W], dt, kind="ExternalOutput").ap()

    AluOp = mybir.AluOpType

    with TileContext(nc) as tc:
        with (
            tc.tile_pool(name="consts", bufs=1) as consts,
            tc.tile_pool(name="work", bufs=2) as work,
            tc.tile_pool(name="acc", bufs=1) as accp,
            tc.tile_pool(name="psum", bufs=1, space="PSUM") as psump,
        ):
            ident = consts.tile([NG * 2 * HP, 2 * HP], dt)
            nc.gpsimd.dma_start(out=ident[:], in_=ident_in[:])

            cm = consts.tile([128, 4], dt)
            cm_ap = cm[:]
            modd_dst = bass.AP(tensor=cm_ap.tensor, offset=cm_ap.offset + 1,
                               ap=[list(cm_ap.ap[0]), [2, 2]])
            nc.gpsimd.dma_start(out=modd_dst, in_=moddt_in[:])

            macc = accp.tile([NP, IW], dt)
            macc2 = accp.tile([NP, IW], dt)

            for ht in range(NHT):
                fpk = work.tile([NP, PW + WIN], dt, tag="fpk")
                nc.sync.dma_start(
                    out=fpk[:], in_=fwin_in[ht * 128:(ht + 1) * 128, :])
                gpk = fpk[:, PW:PW + WIN]

                for icc in range(NIC):
                    i0 = icc * IC
                    tmp = work.tile([NP, IC * WIN], dt, tag="tmp")
                    fpk_ap = fpk[:]
                    in0 = bass.AP(
                        tensor=fpk_ap.tensor,
                        offset=fpk_ap.offset + i0,
                        ap=[list(fpk_ap.ap[0]), [1, IC], [1, WIN]],
                    )
                    in1 = gpk[:, None, :].broadcast_to([NP, IC, WIN])
                    tmp3 = tmp[:].rearrange("p (i d) -> p i d", d=WIN)
                    nc.vector.tensor_tensor(out=tmp3, in0=in0, in1=in1,
                                            op=AluOp.add)
                    dst = macc if ht == 0 else macc2
                    nc.vector.tensor_reduce(
                        out=dst[:, i0:i0 + IC], in_=tmp3,
                        axis=mybir.AxisListType.X, op=AluOp.min,
                    )
                if ht > 0:
                    nc.vector.tensor_tensor(out=macc[:], in0=macc[:],
                                            in1=macc2[:], op=AluOp.min)

            pt = psump.tile([128, 2 * HP], dt)
            for j in range(NG):
                nc.tensor.matmul(
                    pt[j * IW:(j + 1) * IW, :],
                    macc[j * 2 * HP:(j + 1) * 2 * HP, :],
                    ident[j * 2 * HP:(j + 1) * 2 * HP, :],
                    start=True, stop=True,
                )

            cm_ev = bass.AP(tensor=cm_ap.tensor, offset=cm_ap.offset,
                            ap=[list(cm_ap.ap[0]), [2, 2]])
            pt_ap = pt[:]
            pt3 = bass.AP(tensor=pt_ap.tensor, offset=pt_ap.offset,
                          ap=[list(pt_ap.ap[0]), [HP, 2], [1, HP]])
            nc.vector.tensor_reduce(out=cm_ev, in_=pt3,
                                    axis=mybir.AxisListType.X,
                                    op=AluOp.min, negate=True)

            for ih in range(2):
                outt = work.tile([128, _W], dt, tag="outt")
                src = bass.AP(tensor=cm_ap.tensor,
                              offset=cm_ap.offset + 2 * ih,
                              ap=[list(cm_ap.ap[0]), [0, _W // 2], [1, 2]])
                outt_ap = outt[:]
                dst = bass.AP(tensor=outt_ap.tensor, offset=outt_ap.offset,
                              ap=[list(outt_ap.ap[0]), [2, _W // 2], [1, 2]])
                nc.vector.tensor_copy(dst, src)
                eng = nc.sync if ih == 0 else nc.scalar
                eng.dma_start(out=out_ext[ih * 128:(ih + 1) * 128, :],
                              in_=outt[:])

    nc.compile()
    return nc


_USE_RAW = True


def _get_bass(R):
    key = ("fast", R) if R <= _R_FAST_MAX else ("slow", R)
    if key not in _KERNEL_CACHE:
        if key[0] == "fast":
            builder = _build_bass_raw if _USE_RAW else _build_bass_fast
        else:
            builder = _build_bass
        _KERNEL_CACHE[key] = builder(R)
    return _KERNEL_CACHE[key]


def _modd_vec():
    ii = np.arange(_H)
    return np.sqrt(
        np.float32(255.0) ** 2
        + np.maximum(ii, 255 - ii).astype(np.float32) ** 2
    ).astype(np.float32)


def kernel(feature_map, feature_size=None, **_unused):
    from concourse.bass_utils import run_bass_kernel_spmd

    f = np.ascontiguousarray(np.asarray(feature_map, dtype=np.float32))
    assert f.shape == (_B, 1, _H, _W), f.shape

    # Exact pruning radius. The window min for (b,i) is upper-bounded by
    # U[b,i] = min_{|d|<=r0} (f[b,0,0,i+d] + |d|)  (witnesses at h=0, and
    # |d| <= D there), while every point with max(h,|i-w|) >= R is
    # >= R + fmin.  So R >= ceil(max U - fmin) + 1 keeps the windowed min
    # EXACTLY equal to the global min (+1 also covers bf16 rounding).
    fmin = float(f.min())
    f0 = f[:, 0, 0, :]

    def _ubound(r0):
        U = np.full_like(f0, np.inf)
        for d in range(-r0, r0 + 1):
            lo, hi = max(0, -d), _W - max(0, d)
            U[:, lo:hi] = np.minimum(U[:, lo:hi],
                                     f0[:, lo + d:hi + d] + abs(d))
        return float(U.max())

    r0 = 4
    R = int(np.ceil(_ubound(r0) - fmin)) + 1
    if R - 1 < r0:
        # U's witnesses must lie inside the window: redo with a valid r0
        R = int(np.ceil(_ubound(max(0, R - 1)) - fmin)) + 1
    R = max(2, min(R, _H))

    modd = _modd_vec()
    nc = _get_bass(R)

    if R <= _R_FAST_MAX:
        WIN = 2 * R - 1
        hh = np.arange(R, dtype=np.float32)
        dd = np.arange(-(R - 1), R, dtype=np.float32)
        gtab = np.sqrt(hh[:, None] ** 2 + dd[None, :] ** 2).astype(np.float32)
        gtab_row = gtab.reshape(1, R * WIN)
        moddt = np.ascontiguousarray(modd.reshape(2, 128).T)
        in_maps = [{"pk": _pack_fast(f[b, 0], R, gtab_row), "moddt": moddt}
                   for b in range(_B)]
    else:
        G, HP, NHT, NG, IW, WIN, PW, W2, IC = _params(R)
        hh = np.arange(NHT * HP, dtype=np.float32)
        dd = np.arange(-(R - 1), R, dtype=np.float32)
        gtab = np.sqrt(hh[:, None] ** 2 + dd[None, :] ** 2).astype(np.float32)
        gtab[R:, :] = 0.0
        gdup = np.concatenate([np.tile(gtab[t * HP:(t + 1) * HP], (G, 1))
                               for t in range(NHT)], axis=0)
        moddt = np.ascontiguousarray(modd.reshape(2, 128).T)
        ident = np.ascontiguousarray(
            np.tile(np.eye(2 * HP, dtype=np.float32), (NG, 1)))
        in_maps = []
        for b in range(_B):
            fw = np.full((NHT * HP, W2), _PAD, np.float32)
            fw[:R, R - 1:R - 1 + _W] = f[b, 0, :R, :]
            fpk = np.empty((NHT, 128, PW + WIN), np.float32)
            for j in range(NG):
                for ih in range(2):
                    ib = ih * NG + j
                    p0 = j * 2 * HP + ih * HP
                    for t in range(NHT):
                        fpk[t, p0:p0 + HP, :PW] = \
                            fw[t * HP:(t + 1) * HP, ib * IW:ib * IW + PW]
            fpk[:, :, PW:] = gdup.reshape(NHT, 128, WIN)
            fpk = np.ascontiguousarray(fpk.reshape(NHT * 128, PW + WIN))
            in_maps.append({"fwin": fpk, "moddt": moddt, "ident": ident})

    res = run_bass_kernel_spmd(nc, in_maps, list(range(_N_CORES)))
    out = np.stack([res.results[b]["out"] for b in range(_B)])[:, None]
    return np.ascontiguousarray(out.astype(np.float32))
